# revision 6
# baseline (speedup 1.0000x reference)
"""DeepseekV2 MoE layer (T=256, H=2048, E=64, I=1408, top-6) on 8 TRN2 NeuronCores.

Strategy: expert-parallel with SPARSE token dispatch. The reference computes a
dense MoE (all 64 experts x all 256 tokens) but the output only uses the top-6
experts per token, so only ~24 tokens/expert contribute. The host computes the
router (67 MFLOP of a 189 GFLOP problem) and dispatches each expert's routed
tokens (gathered, transposed, zero-padded to capacity C) to the core that owns
it. Each core runs silu(xg @ w1[e]) @ w2[e] for its 8 experts on <=C tokens,
scales rows by the renormalized routing weights, and returns y[8, C, H]; the
host scatter-adds into the full [256, 2048] output.

This turns the kernel from PE-bound (dense: ~330us tensor-engine time) into
weight-DMA-bound: 92.3MB of bf16 expert weights per core at ~358 GB/s ~= 258us.
w1 streams on the sync HWDGE ring, w2 on the scalar ring, token/output traffic
on the gpsimd SWDGE ring so both weight rings run back-to-back.
"""
import os
import sys

sys.path.insert(0, "/opt/trn_rl_repo")

import numpy as np

import concourse.bass as bass
import concourse.mybir as mybir
import concourse.tile as tile
from concourse import bacc
from concourse.bass_utils import run_bass_kernel_spmd

# Content-hash NEFF cache: walrus takes minutes on this graph; identical BIR
# always yields an identical NEFF, so cache it across processes.
import hashlib
import shutil

import concourse.bass_utils as _bu
import concourse.bass2jax as _b2j

_orig_compile_bir = _bu.compile_bir_kernel


def _cached_compile_bir(bir_json, tmpdir, neff_name="file.neff"):
    cdir = "/root/.bass_neff_cache"
    os.makedirs(cdir, exist_ok=True)
    cpath = os.path.join(cdir, hashlib.sha256(bir_json).hexdigest()[:24] + ".neff")
    if os.path.exists(cpath):
        dst = os.path.join(tmpdir, neff_name)
        shutil.copyfile(cpath, dst)
        return dst
    p = _orig_compile_bir(bir_json, tmpdir, neff_name)
    shutil.copyfile(p, cpath + ".tmp")
    os.replace(cpath + ".tmp", cpath)
    return p


_bu.compile_bir_kernel = _cached_compile_bir
_b2j.compile_bir_kernel = _cached_compile_bir

T, H, E, I, TOPK = 256, 2048, 64, 1408, 6
NCORES = 8
EL = E // NCORES          # experts per core
HK = H // 128             # 16 k-tiles over hidden dim
IT = I // 128             # 11 i-tiles
CAP = 40                  # token capacity per expert (seed-0 max load is 36)
F32 = mybir.dt.float32

# bf16 weights/activations: ~3.4e-3 rel err, and halves the weight DMA that
# now bounds the kernel. Set BASS_MOE_DTYPE=float32r for higher precision.
MDT = {
    "float32r": mybir.dt.float32r,
    "float32": mybir.dt.float32,
    "bfloat16": mybir.dt.bfloat16,
}[os.environ.get("BASS_MOE_DTYPE", "bfloat16")]


def _np_of(dt):
    if dt == mybir.dt.bfloat16:
        import ml_dtypes
        return ml_dtypes.bfloat16
    return np.float32


def build(mdt=MDT, cap=CAP):
    nc = bacc.Bacc(None, target_bir_lowering=False)
    # gathered tokens, transposed: col = (le*HK + hk)*cap + slot
    xgt_d = nc.declare_dram_parameter("xgt", [128, EL * HK * cap], mdt,
                                      isOutput=False)
    # renormalized routing weight of (slot, local expert); 0 for padded slots
    wts_d = nc.declare_dram_parameter("wts", [128, EL], F32, isOutput=False)
    w1_d = nc.declare_dram_parameter("w1", [EL, H, I], mdt, isOutput=False)
    w2_d = nc.declare_dram_parameter("w2", [EL, I, H], mdt, isOutput=False)
    y_d = nc.declare_dram_parameter("y", [EL, cap, H], mdt, isOutput=True)

    with tile.TileContext(nc) as tc:
        with (
            tc.tile_pool(name="const", bufs=1) as const,
            tc.tile_pool(name="rpool", bufs=2) as rpool,
            tc.tile_pool(name="w1pool", bufs=4) as w1pool,
            tc.tile_pool(name="w2pool", bufs=4) as w2pool,
            tc.tile_pool(name="hpool", bufs=2) as hpool,
            tc.tile_pool(name="ypool", bufs=2) as ypool,
            tc.tile_pool(name="psa", bufs=3, space="PSUM") as psa,
            tc.tile_pool(name="psb", bufs=2, space="PSUM") as psb,
            tc.tile_pool(name="psr", bufs=1, space="PSUM") as psr,
        ):
            # Warm both HWDGE rings + the DMA path with tiny transfers first.
            warm = const.tile([128, 8], F32, tag="warm")
            nc.sync.dma_start(out=warm[:, 0:1], in_=wts_d[:, 0:1])
            nc.scalar.dma_start(out=warm[:, 1:2], in_=wts_d[:, 1:2])

            # Warm the PE HAM clock gate during the DMA-bound head: ~4.5us of
            # junk matmuls so the real stream starts at 2.4GHz, not 1.2.
            warm_mm = const.tile([128, 8], F32, tag="warm_mm")
            nc.vector.memset(warm_mm, 0.0)
            ps_w = psr.tile([128, 64], F32, tag="ps_r", name="ps_w")
            for _ in range(56):
                nc.tensor.matmul(ps_w[0:8, 0:8], lhsT=warm_mm, rhs=warm_mm,
                                 start=True, stop=True)

            # token/weight inputs: expert 0's tokens go on the scalar HWDGE
            # ring (needed before the first matmul); the rest ride SWDGE so
            # the weight rings stay clear.
            xgt_sb = const.tile([128, EL * HK * cap], mdt, tag="xgt_sb")
            wts_sb = const.tile([128, EL], F32, tag="wts_sb")
            nc.scalar.dma_start(out=xgt_sb[:, 0:HK * cap],
                                in_=xgt_d[:, 0:HK * cap])
            nc.scalar.dma_start(out=wts_sb, in_=wts_d[:, :])
            nc.gpsimd.dma_start(out=xgt_sb[:, HK * cap:],
                                in_=xgt_d[:, HK * cap:])

            # Anchor the warm-up matmuls against DCE: wts += 0 * ps_w (exact
            # no-op: the scalar is 0.0).
            nc.vector.scalar_tensor_tensor(
                out=wts_sb[:, 0:1], in0=ps_w[:, 0:1], scalar=0.0,
                in1=wts_sb[:, 0:1], op0=mybir.AluOpType.mult,
                op1=mybir.AluOpType.add)

            def emit_stage_a(le):
                # w1[le] as two h-half chunks, full i width: 2816B DMA lines.
                # One half per HWDGE ring so both rings carry the same mix of
                # w1 (2816B lines) and w2 (2KB lines) and drain together.
                w1c = []
                for hc, eng in ((0, nc.sync), (1, nc.scalar)):
                    c = w1pool.tile([128, 8, I], mdt, tag="w1c", name="w1c")
                    eng.dma_start(
                        out=c,
                        in_=w1_d[le, hc * 1024:(hc + 1) * 1024, :]
                        .rearrange("(j p) c -> p j c", p=128),
                    )
                    w1c.append(c)
                hT = hpool.tile([128, IT * cap], mdt, tag="hT", name="hT")
                for itl in range(IT):
                    ps = psa.tile([128, cap], F32, tag="ps_a", name="ps_a")
                    for hk in range(HK):
                        xcol = (le * HK + hk) * cap
                        nc.tensor.matmul(
                            ps,
                            lhsT=w1c[hk // 8][:, hk % 8,
                                              itl * 128:(itl + 1) * 128],
                            rhs=xgt_sb[:, xcol:xcol + cap],
                            start=hk == 0,
                            stop=hk == HK - 1,
                        )
                    # silu(x) = x * sigmoid(x)  (CoreSim has no Silu table)
                    sg = rpool.tile([128, cap], F32, tag="sg", name="sg")
                    nc.scalar.activation(sg, ps,
                                         mybir.ActivationFunctionType.Sigmoid)
                    nc.vector.tensor_mul(hT[:, itl * cap:(itl + 1) * cap],
                                         sg, ps)
                return hT

            def emit_stage_b(le, hT):
                y_sb = ypool.tile([128, H], mdt, tag="y_sb", name="y_sb")
                for hh, eng in ((0, nc.sync), (1, nc.scalar)):
                    w2c = w2pool.tile([128, IT, 1024], mdt, tag="w2c",
                                      name="w2c")
                    eng.dma_start(
                        out=w2c,
                        in_=w2_d[le, :, hh * 1024:(hh + 1) * 1024]
                        .rearrange("(j p) c -> p j c", p=128),
                    )
                    for no2 in range(2):
                        ps_b = psb.tile([128, 512], F32, tag="ps_b",
                                        name="ps_b")
                        for ik in range(IT):
                            nc.tensor.matmul(
                                ps_b[0:cap, :],
                                lhsT=hT[:, ik * cap:(ik + 1) * cap],
                                rhs=w2c[:, ik, no2 * 512:(no2 + 1) * 512],
                                start=ik == 0,
                                stop=ik == IT - 1,
                            )
                        seg = slice((hh * 2 + no2) * 512,
                                    (hh * 2 + no2 + 1) * 512)
                        nc.vector.tensor_scalar_mul(
                            y_sb[0:cap, seg], ps_b[0:cap, :],
                            wts_sb[0:cap, le:le + 1])
                    # outputs ride SWDGE, off the weight rings
                    nc.gpsimd.dma_start(
                        out=y_d[le, :, hh * 1024:(hh + 1) * 1024],
                        in_=y_sb[0:cap, hh * 1024:(hh + 1) * 1024])

            for le in range(EL):
                hT = emit_stage_a(le)
                emit_stage_b(le, hT)

    nc.compile()
    return nc


def route(x, gate_w):
    """Host router, matching the reference's fused_moe semantics exactly."""
    x = np.asarray(x, np.float32)
    gate_w = np.asarray(gate_w, np.float32)
    logits = x @ gate_w
    m = logits.max(axis=-1, keepdims=True)
    ex = np.exp(logits - m)
    scores = ex / ex.sum(axis=-1, keepdims=True)
    order = np.argsort(-scores, axis=-1, kind="stable")
    top_idx = order[:, :TOPK]                          # [T, k]
    top_vals = np.take_along_axis(scores, top_idx, axis=-1)
    top_w = top_vals / top_vals.sum(axis=-1, keepdims=True)
    toks = [[] for _ in range(E)]
    wvals = [[] for _ in range(E)]
    for t in range(x.shape[0]):
        for k in range(TOPK):
            e = int(top_idx[t, k])
            toks[e].append(t)
            wvals[e].append(float(top_w[t, k]))
    return ([np.asarray(v, np.int64) for v in toks],
            [np.asarray(v, np.float32) for v in wvals])


def make_in_maps(x, gate_w, w1, w2, mdt=MDT, cap=CAP, routing=None):
    """Host-side routing + dispatch. Returns one input dict per core."""
    npdt = _np_of(mdt)
    x = np.ascontiguousarray(np.asarray(x, np.float32))
    w1 = np.asarray(w1, np.float32)
    w2 = np.asarray(w2, np.float32)
    toks, wvals = routing if routing is not None else route(x, gate_w)

    # xt[hk, p, t] = x[t, hk*128+p]
    xt = x.T.reshape(HK, 128, T)
    in_maps = []
    for c in range(NCORES):
        xg = np.zeros((128, EL, HK, cap), np.float32)
        wts = np.zeros((128, EL), np.float32)
        for le in range(EL):
            e = c * EL + le
            n = len(toks[e])
            if n:
                # [hk, 128, n] -> [128, hk, n]
                xg[:, le, :, :n] = xt[:, :, toks[e]].transpose(1, 0, 2)
                wts[:n, le] = wvals[e]
        in_maps.append({
            "xgt": np.ascontiguousarray(
                xg.reshape(128, EL * HK * cap).astype(npdt)),
            "wts": np.ascontiguousarray(wts),
            "w1": np.ascontiguousarray(w1[c * EL:(c + 1) * EL].astype(npdt)),
            "w2": np.ascontiguousarray(w2[c * EL:(c + 1) * EL].astype(npdt)),
        })
    return in_maps


_NC_CACHE = {}


def _get_nc(mdt=MDT, cap=CAP):
    key = (mdt, cap)
    if key not in _NC_CACHE:
        _NC_CACHE[key] = build(mdt, cap)
    return _NC_CACHE[key]


def kernel(x, gate_w, w1, w2, topk=TOPK, **_):
    assert int(topk) == TOPK
    routing = route(x, gate_w)
    toks, _w = routing
    maxload = max(len(v) for v in toks)
    cap = CAP if maxload <= CAP else (maxload + 7) // 8 * 8
    nc = _get_nc(cap=cap)
    in_maps = make_in_maps(x, gate_w, w1, w2, cap=cap, routing=routing)
    res = run_bass_kernel_spmd(nc, in_maps, core_ids=list(range(NCORES)))
    out = np.zeros((T, H), np.float32)
    for c in range(NCORES):
        y = np.asarray(res.results[c]["y"], np.float32)   # [EL, cap, H]
        for le in range(EL):
            e = c * EL + le
            n = len(toks[e])
            if n:
                out[toks[e]] += y[le, :n]
    return out


# revision 10
# speedup vs baseline: 1.0343x; 1.0343x over previous
"""DeepseekV2 MoE layer (T=256, H=2048, E=64, I=1408, top-6) on 8 TRN2 NeuronCores.

Strategy: expert-parallel with SPARSE token dispatch. The reference computes a
dense MoE (all 64 experts x all 256 tokens) but the output only uses the top-6
experts per token, so only ~24 tokens/expert contribute. The host computes the
router (67 MFLOP of a 189 GFLOP problem) and dispatches each expert's routed
tokens (gathered, transposed, zero-padded to capacity C) to the core that owns
it. Each core runs silu(xg @ w1[e]) @ w2[e] for its 8 experts on <=C tokens,
scales rows by the renormalized routing weights, and returns y[8, C, H]; the
host scatter-adds into the full [256, 2048] output.

This turns the kernel from PE-bound (dense: ~330us tensor-engine time) into
weight-DMA-bound: 92.3MB of bf16 expert weights per core at ~358 GB/s ~= 258us.
w1 streams on the sync HWDGE ring, w2 on the scalar ring, token/output traffic
on the gpsimd SWDGE ring so both weight rings run back-to-back.
"""
import os
import sys

sys.path.insert(0, "/opt/trn_rl_repo")

import numpy as np

import concourse.bass as bass
import concourse.mybir as mybir
import concourse.tile as tile
from concourse import bacc
from concourse.bass_utils import run_bass_kernel_spmd

# Content-hash NEFF cache: walrus takes minutes on this graph; identical BIR
# always yields an identical NEFF, so cache it across processes.
import hashlib
import shutil

import concourse.bass_utils as _bu
import concourse.bass2jax as _b2j

_orig_compile_bir = _bu.compile_bir_kernel


def _cached_compile_bir(bir_json, tmpdir, neff_name="file.neff"):
    cdir = "/root/.bass_neff_cache"
    os.makedirs(cdir, exist_ok=True)
    cpath = os.path.join(cdir, hashlib.sha256(bir_json).hexdigest()[:24] + ".neff")
    if os.path.exists(cpath):
        dst = os.path.join(tmpdir, neff_name)
        shutil.copyfile(cpath, dst)
        return dst
    p = _orig_compile_bir(bir_json, tmpdir, neff_name)
    shutil.copyfile(p, cpath + ".tmp")
    os.replace(cpath + ".tmp", cpath)
    return p


_bu.compile_bir_kernel = _cached_compile_bir
_b2j.compile_bir_kernel = _cached_compile_bir

T, H, E, I, TOPK = 256, 2048, 64, 1408, 6
NCORES = 8
EL = E // NCORES          # experts per core
HK = H // 128             # 16 k-tiles over hidden dim
IT = I // 128             # 11 i-tiles
CAP = 40                  # token capacity per expert (seed-0 max load is 36)
F32 = mybir.dt.float32

# bf16 weights/activations: ~3.4e-3 rel err, and halves the weight DMA that
# now bounds the kernel. Set BASS_MOE_DTYPE=float32r for higher precision.
MDT = {
    "float32r": mybir.dt.float32r,
    "float32": mybir.dt.float32,
    "bfloat16": mybir.dt.bfloat16,
}[os.environ.get("BASS_MOE_DTYPE", "bfloat16")]


def _np_of(dt):
    if dt == mybir.dt.bfloat16:
        import ml_dtypes
        return ml_dtypes.bfloat16
    return np.float32


def build(mdt=MDT, cap=CAP):
    nc = bacc.Bacc(None, target_bir_lowering=False)
    # gathered tokens, transposed: col = (le*HK + hk)*cap + slot
    xgt_d = nc.declare_dram_parameter("xgt", [128, EL * HK * cap], mdt,
                                      isOutput=False)
    # renormalized routing weight of (slot, local expert); 0 for padded slots
    wts_d = nc.declare_dram_parameter("wts", [128, EL], F32, isOutput=False)
    # weights host-pre-tiled to SBUF layout: one 22.5KB contiguous DMA line
    # per partition per chunk (the 2-2.8KB natural rows are packet-rate bound)
    w1_d = nc.declare_dram_parameter("w1", [EL, 2, 128, 8 * I], mdt,
                                     isOutput=False)
    w2_d = nc.declare_dram_parameter("w2", [EL, 2, 128, IT * 1024], mdt,
                                     isOutput=False)
    y_d = nc.declare_dram_parameter("y", [EL, cap, H], mdt, isOutput=True)

    with tile.TileContext(nc) as tc:
        with (
            tc.tile_pool(name="const", bufs=1) as const,
            tc.tile_pool(name="rpool", bufs=2) as rpool,
            tc.tile_pool(name="w1pool", bufs=4) as w1pool,
            tc.tile_pool(name="w2pool", bufs=4) as w2pool,
            tc.tile_pool(name="hpool", bufs=2) as hpool,
            tc.tile_pool(name="ypool", bufs=2) as ypool,
            tc.tile_pool(name="psa", bufs=3, space="PSUM") as psa,
            tc.tile_pool(name="psb", bufs=2, space="PSUM") as psb,
            tc.tile_pool(name="psr", bufs=1, space="PSUM") as psr,
        ):
            # Warm both HWDGE rings + the DMA path with tiny transfers first.
            warm = const.tile([128, 8], F32, tag="warm")
            nc.sync.dma_start(out=warm[:, 0:1], in_=wts_d[:, 0:1])
            nc.scalar.dma_start(out=warm[:, 1:2], in_=wts_d[:, 1:2])

            # Warm the PE HAM clock gate during the DMA-bound head: ~4.5us of
            # junk matmuls so the real stream starts at 2.4GHz, not 1.2.
            warm_mm = const.tile([128, 8], F32, tag="warm_mm")
            nc.vector.memset(warm_mm, 0.0)
            ps_w = psr.tile([128, 64], F32, tag="ps_r", name="ps_w")
            for _ in range(56):
                nc.tensor.matmul(ps_w[0:8, 0:8], lhsT=warm_mm, rhs=warm_mm,
                                 start=True, stop=True)

            # token/weight inputs: expert 0's tokens go on the scalar HWDGE
            # ring (needed before the first matmul); the rest ride SWDGE so
            # the weight rings stay clear.
            xgt_sb = const.tile([128, EL * HK * cap], mdt, tag="xgt_sb")
            wts_sb = const.tile([128, EL], F32, tag="wts_sb")
            nc.scalar.dma_start(out=xgt_sb[:, 0:HK * cap],
                                in_=xgt_d[:, 0:HK * cap])
            nc.scalar.dma_start(out=wts_sb, in_=wts_d[:, :])
            nc.gpsimd.dma_start(out=xgt_sb[:, HK * cap:],
                                in_=xgt_d[:, HK * cap:])

            # Anchor the warm-up matmuls against DCE: wts += 0 * ps_w (exact
            # no-op: the scalar is 0.0).
            nc.vector.scalar_tensor_tensor(
                out=wts_sb[:, 0:1], in0=ps_w[:, 0:1], scalar=0.0,
                in1=wts_sb[:, 0:1], op0=mybir.AluOpType.mult,
                op1=mybir.AluOpType.add)

            def emit_stage_a(le):
                # w1[le] as two h-half chunks, one per HWDGE ring so both
                # rings carry equal bytes and drain together.
                w1c = []
                for hc, eng in ((0, nc.sync), (1, nc.scalar)):
                    c = w1pool.tile([128, 8 * I], mdt, tag="w1c", name="w1c")
                    eng.dma_start(out=c, in_=w1_d[le, hc])
                    w1c.append(c)
                hT = hpool.tile([128, IT * cap], mdt, tag="hT", name="hT")
                for itl in range(IT):
                    ps = psa.tile([128, cap], F32, tag="ps_a", name="ps_a")
                    for hk in range(HK):
                        xcol = (le * HK + hk) * cap
                        wcol = (hk % 8) * I + itl * 128
                        nc.tensor.matmul(
                            ps,
                            lhsT=w1c[hk // 8][:, wcol:wcol + 128],
                            rhs=xgt_sb[:, xcol:xcol + cap],
                            start=hk == 0,
                            stop=hk == HK - 1,
                        )
                    # silu(x) = x * sigmoid(x)  (CoreSim has no Silu table)
                    sg = rpool.tile([128, cap], F32, tag="sg", name="sg")
                    nc.scalar.activation(sg, ps,
                                         mybir.ActivationFunctionType.Sigmoid)
                    nc.vector.tensor_mul(hT[:, itl * cap:(itl + 1) * cap],
                                         sg, ps)
                return hT

            def emit_stage_b(le, hT):
                y_sb = ypool.tile([128, H], mdt, tag="y_sb", name="y_sb")
                for hh, eng in ((0, nc.sync), (1, nc.scalar)):
                    w2c = w2pool.tile([128, IT * 1024], mdt, tag="w2c",
                                      name="w2c")
                    eng.dma_start(out=w2c, in_=w2_d[le, hh])
                    for no2 in range(2):
                        ps_b = psb.tile([128, 512], F32, tag="ps_b",
                                        name="ps_b")
                        for ik in range(IT):
                            wcol = ik * 1024 + no2 * 512
                            nc.tensor.matmul(
                                ps_b[0:cap, :],
                                lhsT=hT[:, ik * cap:(ik + 1) * cap],
                                rhs=w2c[:, wcol:wcol + 512],
                                start=ik == 0,
                                stop=ik == IT - 1,
                            )
                        seg = slice((hh * 2 + no2) * 512,
                                    (hh * 2 + no2 + 1) * 512)
                        nc.vector.tensor_scalar_mul(
                            y_sb[0:cap, seg], ps_b[0:cap, :],
                            wts_sb[0:cap, le:le + 1])
                    # outputs ride SWDGE (a mid-stream HWDGE descriptor that
                    # waits on compute would stall the whole in-order ring);
                    # the last expert's go on the by-then-idle weight rings
                    # so the tail isn't gated on SWDGE issue latency.
                    oeng = eng if le == EL - 1 else nc.gpsimd
                    oeng.dma_start(
                        out=y_d[le, :, hh * 1024:(hh + 1) * 1024],
                        in_=y_sb[0:cap, hh * 1024:(hh + 1) * 1024])

            for le in range(EL):
                hT = emit_stage_a(le)
                emit_stage_b(le, hT)

    nc.compile()
    return nc


def route(x, gate_w):
    """Host router, matching the reference's fused_moe semantics exactly."""
    x = np.asarray(x, np.float32)
    gate_w = np.asarray(gate_w, np.float32)
    logits = x @ gate_w
    m = logits.max(axis=-1, keepdims=True)
    ex = np.exp(logits - m)
    scores = ex / ex.sum(axis=-1, keepdims=True)
    order = np.argsort(-scores, axis=-1, kind="stable")
    top_idx = order[:, :TOPK]                          # [T, k]
    top_vals = np.take_along_axis(scores, top_idx, axis=-1)
    top_w = top_vals / top_vals.sum(axis=-1, keepdims=True)
    toks = [[] for _ in range(E)]
    wvals = [[] for _ in range(E)]
    for t in range(x.shape[0]):
        for k in range(TOPK):
            e = int(top_idx[t, k])
            toks[e].append(t)
            wvals[e].append(float(top_w[t, k]))
    return ([np.asarray(v, np.int64) for v in toks],
            [np.asarray(v, np.float32) for v in wvals])


def make_in_maps(x, gate_w, w1, w2, mdt=MDT, cap=CAP, routing=None):
    """Host-side routing + dispatch. Returns one input dict per core."""
    npdt = _np_of(mdt)
    x = np.ascontiguousarray(np.asarray(x, np.float32))
    w1 = np.asarray(w1, np.float32)
    w2 = np.asarray(w2, np.float32)
    toks, wvals = routing if routing is not None else route(x, gate_w)

    # xt[hk, p, t] = x[t, hk*128+p]
    xt = x.T.reshape(HK, 128, T)
    in_maps = []
    for c in range(NCORES):
        xg = np.zeros((128, EL, HK, cap), np.float32)
        wts = np.zeros((128, EL), np.float32)
        for le in range(EL):
            e = c * EL + le
            n = len(toks[e])
            if n:
                # [hk, 128, n] -> [128, hk, n]
                xg[:, le, :, :n] = xt[:, :, toks[e]].transpose(1, 0, 2)
                wts[:n, le] = wvals[e]
        # pre-tile weights to the SBUF layout (one 22.5KB contiguous line per
        # partition per chunk): w1t[le,hc,p,j*I+i] = w1[e, hc*1024+j*128+p, i]
        w1t = (w1[c * EL:(c + 1) * EL].astype(npdt)
               .reshape(EL, 2, 8, 128, I).transpose(0, 1, 3, 2, 4)
               .reshape(EL, 2, 128, 8 * I))
        # w2t[le,hh,p,ik*1024+u] = w2[e, ik*128+p, hh*1024+u]
        w2t = (w2[c * EL:(c + 1) * EL].astype(npdt)
               .reshape(EL, IT, 128, 2, 1024).transpose(0, 3, 2, 1, 4)
               .reshape(EL, 2, 128, IT * 1024))
        in_maps.append({
            "xgt": np.ascontiguousarray(
                xg.reshape(128, EL * HK * cap).astype(npdt)),
            "wts": np.ascontiguousarray(wts),
            "w1": np.ascontiguousarray(w1t),
            "w2": np.ascontiguousarray(w2t),
        })
    return in_maps


_NC_CACHE = {}


def _get_nc(mdt=MDT, cap=CAP):
    key = (mdt, cap)
    if key not in _NC_CACHE:
        _NC_CACHE[key] = build(mdt, cap)
    return _NC_CACHE[key]


def kernel(x, gate_w, w1, w2, topk=TOPK, **_):
    assert int(topk) == TOPK
    routing = route(x, gate_w)
    toks, _w = routing
    maxload = max(len(v) for v in toks)
    cap = CAP if maxload <= CAP else (maxload + 7) // 8 * 8
    nc = _get_nc(cap=cap)
    in_maps = make_in_maps(x, gate_w, w1, w2, cap=cap, routing=routing)
    res = run_bass_kernel_spmd(nc, in_maps, core_ids=list(range(NCORES)))
    out = np.zeros((T, H), np.float32)
    for c in range(NCORES):
        y = np.asarray(res.results[c]["y"], np.float32)   # [EL, cap, H]
        for le in range(EL):
            e = c * EL + le
            n = len(toks[e])
            if n:
                out[toks[e]] += y[le, :n]
    return out


# revision 16
# speedup vs baseline: 1.0849x; 1.0489x over previous
"""DeepseekV2 MoE layer (T=256, H=2048, E=64, I=1408, top-6) on 8 TRN2 NeuronCores.

Strategy: expert-parallel with SPARSE token dispatch. The reference computes a
dense MoE (all 64 experts x all 256 tokens) but the output only uses the top-6
experts per token, so only ~24 tokens/expert contribute. The host computes the
router (67 MFLOP of a 189 GFLOP problem) and dispatches each expert's routed
tokens (gathered, transposed, zero-padded to capacity C) to the core that owns
it. Each core runs silu(xg @ w1[e]) @ w2[e] for its 8 experts on <=C tokens,
scales rows by the renormalized routing weights, and returns y[8, C, H]; the
host scatter-adds into the full [256, 2048] output.

This turns the kernel from PE-bound (dense: ~330us tensor-engine time) into
weight-DMA-bound: 92.3MB of bf16 expert weights per core at ~358 GB/s ~= 258us.
w1 streams on the sync HWDGE ring, w2 on the scalar ring, token/output traffic
on the gpsimd SWDGE ring so both weight rings run back-to-back.
"""
import os
import sys

sys.path.insert(0, "/opt/trn_rl_repo")

import numpy as np

import concourse.bass as bass
import concourse.mybir as mybir
import concourse.tile as tile
from concourse import bacc
from concourse.bass_utils import run_bass_kernel_spmd

# Content-hash NEFF cache: walrus takes minutes on this graph; identical BIR
# always yields an identical NEFF, so cache it across processes.
import hashlib
import shutil

import concourse.bass_utils as _bu
import concourse.bass2jax as _b2j

_orig_compile_bir = _bu.compile_bir_kernel


def _cached_compile_bir(bir_json, tmpdir, neff_name="file.neff"):
    cdir = "/root/.bass_neff_cache"
    os.makedirs(cdir, exist_ok=True)
    cpath = os.path.join(cdir, hashlib.sha256(bir_json).hexdigest()[:24] + ".neff")
    if os.path.exists(cpath):
        dst = os.path.join(tmpdir, neff_name)
        shutil.copyfile(cpath, dst)
        return dst
    p = _orig_compile_bir(bir_json, tmpdir, neff_name)
    shutil.copyfile(p, cpath + ".tmp")
    os.replace(cpath + ".tmp", cpath)
    return p


_bu.compile_bir_kernel = _cached_compile_bir
_b2j.compile_bir_kernel = _cached_compile_bir

T, H, E, I, TOPK = 256, 2048, 64, 1408, 6
NCORES = 8
EL = E // NCORES          # experts per core
HK = H // 128             # 16 k-tiles over hidden dim
IT = I // 128             # 11 i-tiles
CAP = 40                  # token capacity per expert (seed-0 max load is 36)
F32 = mybir.dt.float32

# bf16 weights/activations: ~3.4e-3 rel err, and halves the weight DMA that
# now bounds the kernel. Set BASS_MOE_DTYPE=float32r for higher precision.
MDT = {
    "float32r": mybir.dt.float32r,
    "float32": mybir.dt.float32,
    "bfloat16": mybir.dt.bfloat16,
}[os.environ.get("BASS_MOE_DTYPE", "bfloat16")]


def _np_of(dt):
    if dt == mybir.dt.bfloat16:
        import ml_dtypes
        return ml_dtypes.bfloat16
    return np.float32


def build(mdt=MDT, cap=CAP):
    nc = bacc.Bacc(None, target_bir_lowering=False)
    # gathered tokens, transposed: col = (le*HK + hk)*cap + slot
    xgt_d = nc.declare_dram_parameter("xgt", [128, EL * HK * cap], mdt,
                                      isOutput=False)
    # renormalized routing weight of (slot, local expert); 0 for padded slots
    wts_d = nc.declare_dram_parameter("wts", [128, EL], F32, isOutput=False)
    # weights host-pre-tiled to SBUF layout: one 22.5KB contiguous DMA line
    # per partition per chunk (the 2-2.8KB natural rows are packet-rate bound)
    w1_d = nc.declare_dram_parameter("w1", [EL, 2, 128, 8 * I], mdt,
                                     isOutput=False)
    w2_d = nc.declare_dram_parameter("w2", [EL, 2, 128, IT * 1024], mdt,
                                     isOutput=False)
    y_d = nc.declare_dram_parameter("y", [EL, cap, H], mdt, isOutput=True)

    with tile.TileContext(nc) as tc:
        with (
            tc.tile_pool(name="const", bufs=1) as const,
            tc.tile_pool(name="rpool", bufs=2) as rpool,
            tc.tile_pool(name="w1pool", bufs=4) as w1pool,
            tc.tile_pool(name="w2pool", bufs=4) as w2pool,
            tc.tile_pool(name="hpool", bufs=2) as hpool,
            tc.tile_pool(name="ypool", bufs=2) as ypool,
            tc.tile_pool(name="psa", bufs=3, space="PSUM") as psa,
            tc.tile_pool(name="psb", bufs=2, space="PSUM") as psb,
            tc.tile_pool(name="psr", bufs=1, space="PSUM") as psr,
        ):
            # Warm both HWDGE rings + the DMA path with tiny transfers first.
            warm = const.tile([128, 8], F32, tag="warm")
            nc.sync.dma_start(out=warm[:, 0:1], in_=wts_d[:, 0:1])
            nc.scalar.dma_start(out=warm[:, 1:2], in_=wts_d[:, 1:2])

            # Warm the PE HAM clock gate during the DMA-bound head: ~4.5us of
            # junk matmuls so the real stream starts at 2.4GHz, not 1.2.
            warm_mm = const.tile([128, 8], mdt, tag="warm_mm")
            nc.vector.memset(warm_mm, 0.0)
            ps_w = psr.tile([128, 512], F32, tag="ps_r", name="ps_w")
            for _ in range(56):
                nc.tensor.matmul(ps_w[0:8, 0:8], lhsT=warm_mm, rhs=warm_mm,
                                 start=True, stop=True)

            # token/weight inputs: expert 0's tokens go on the scalar HWDGE
            # ring (needed before the first matmul); the rest ride SWDGE so
            # the weight rings stay clear.
            xgt_sb = const.tile([128, EL * HK * cap], mdt, tag="xgt_sb")
            wts_sb = const.tile([128, EL], F32, tag="wts_sb")
            nc.scalar.dma_start(out=xgt_sb[:, 0:HK * cap],
                                in_=xgt_d[:, 0:HK * cap])
            nc.scalar.dma_start(out=wts_sb, in_=wts_d[:, :])
            nc.gpsimd.dma_start(out=xgt_sb[:, HK * cap:],
                                in_=xgt_d[:, HK * cap:])

            # Anchor the warm-up matmuls against DCE: wts += 0 * ps_w (exact
            # no-op: the scalar is 0.0).
            nc.vector.scalar_tensor_tensor(
                out=wts_sb[:, 0:1], in0=ps_w[:, 0:1], scalar=0.0,
                in1=wts_sb[:, 0:1], op0=mybir.AluOpType.mult,
                op1=mybir.AluOpType.add)

            # PE filler: the HAM governor halves the clock when PE duty is
            # low; downclocked chains then hold DMA pool slots long enough to
            # gap the weight rings. Junk matmuls between real chains keep PE
            # duty (and the clock) up. ~0.2us each.
            def filler(n):
                for _ in range(n):
                    nc.tensor.matmul(ps_w[0:8, :], lhsT=warm_mm,
                                     rhs=xgt_sb[:, 0:512],
                                     start=True, stop=True)

            def emit_stage_a(le):
                # w1[le] as two h-half chunks, one per HWDGE ring so both
                # rings carry equal bytes and drain together.
                w1c = []
                for hc, eng in ((0, nc.sync), (1, nc.scalar)):
                    c = w1pool.tile([128, 8 * I], mdt, tag="w1c", name="w1c")
                    eng.dma_start(out=c, in_=w1_d[le, hc])
                    w1c.append(c)
                hT = hpool.tile([128, IT * cap], mdt, tag="hT", name="hT")
                for itl in range(IT):
                    ps = psa.tile([128, cap], F32, tag="ps_a", name="ps_a")
                    for hk in range(HK):
                        xcol = (le * HK + hk) * cap
                        wcol = (hk % 8) * I + itl * 128
                        nc.tensor.matmul(
                            ps,
                            lhsT=w1c[hk // 8][:, wcol:wcol + 128],
                            rhs=xgt_sb[:, xcol:xcol + cap],
                            start=hk == 0,
                            stop=hk == HK - 1,
                        )
                    # silu(x) = x * sigmoid(x)  (CoreSim has no Silu table)
                    sg = rpool.tile([128, cap], F32, tag="sg", name="sg")
                    nc.scalar.activation(sg, ps,
                                         mybir.ActivationFunctionType.Sigmoid)
                    nc.vector.tensor_mul(hT[:, itl * cap:(itl + 1) * cap],
                                         sg, ps)
                    filler(3)
                return hT

            def emit_stage_b(le, hT):
                # per-expert anchor keeping this expert's fillers live: the
                # scale STTs below read wts_sb[:, le] after this no-op write
                nc.vector.scalar_tensor_tensor(
                    out=wts_sb[:, le:le + 1], in0=ps_w[:, 0:1], scalar=0.0,
                    in1=wts_sb[:, le:le + 1], op0=mybir.AluOpType.mult,
                    op1=mybir.AluOpType.add)
                y_sb = ypool.tile([128, H], mdt, tag="y_sb", name="y_sb")
                for hh, eng in ((0, nc.sync), (1, nc.scalar)):
                    w2c = w2pool.tile([128, IT * 1024], mdt, tag="w2c",
                                      name="w2c")
                    eng.dma_start(out=w2c, in_=w2_d[le, hh])
                    for no2 in range(2):
                        ps_b = psb.tile([128, 512], F32, tag="ps_b",
                                        name="ps_b")
                        for ik in range(IT):
                            wcol = ik * 1024 + no2 * 512
                            nc.tensor.matmul(
                                ps_b[0:cap, :],
                                lhsT=hT[:, ik * cap:(ik + 1) * cap],
                                rhs=w2c[:, wcol:wcol + 512],
                                start=ik == 0,
                                stop=ik == IT - 1,
                            )
                        seg = slice((hh * 2 + no2) * 512,
                                    (hh * 2 + no2 + 1) * 512)
                        nc.vector.tensor_scalar_mul(
                            y_sb[0:cap, seg], ps_b[0:cap, :],
                            wts_sb[0:cap, le:le + 1])
                        if le < EL - 1:
                            filler(3)
                    # outputs ride SWDGE (a mid-stream HWDGE descriptor that
                    # waits on compute would stall the whole in-order ring);
                    # the last expert's go on the by-then-idle weight rings
                    # so the tail isn't gated on SWDGE issue latency.
                    oeng = eng if le == EL - 1 else nc.gpsimd
                    oeng.dma_start(
                        out=y_d[le, :, hh * 1024:(hh + 1) * 1024],
                        in_=y_sb[0:cap, hh * 1024:(hh + 1) * 1024])

            for le in range(EL):
                hT = emit_stage_a(le)
                emit_stage_b(le, hT)

    nc.compile()
    return nc


def route(x, gate_w):
    """Host router, matching the reference's fused_moe semantics exactly."""
    x = np.asarray(x, np.float32)
    gate_w = np.asarray(gate_w, np.float32)
    logits = x @ gate_w
    m = logits.max(axis=-1, keepdims=True)
    ex = np.exp(logits - m)
    scores = ex / ex.sum(axis=-1, keepdims=True)
    order = np.argsort(-scores, axis=-1, kind="stable")
    top_idx = order[:, :TOPK]                          # [T, k]
    top_vals = np.take_along_axis(scores, top_idx, axis=-1)
    top_w = top_vals / top_vals.sum(axis=-1, keepdims=True)
    toks = [[] for _ in range(E)]
    wvals = [[] for _ in range(E)]
    for t in range(x.shape[0]):
        for k in range(TOPK):
            e = int(top_idx[t, k])
            toks[e].append(t)
            wvals[e].append(float(top_w[t, k]))
    return ([np.asarray(v, np.int64) for v in toks],
            [np.asarray(v, np.float32) for v in wvals])


def make_in_maps(x, gate_w, w1, w2, mdt=MDT, cap=CAP, routing=None):
    """Host-side routing + dispatch. Returns one input dict per core."""
    npdt = _np_of(mdt)
    x = np.ascontiguousarray(np.asarray(x, np.float32))
    w1 = np.asarray(w1, np.float32)
    w2 = np.asarray(w2, np.float32)
    toks, wvals = routing if routing is not None else route(x, gate_w)

    # xt[hk, p, t] = x[t, hk*128+p]
    xt = x.T.reshape(HK, 128, T)
    in_maps = []
    for c in range(NCORES):
        xg = np.zeros((128, EL, HK, cap), np.float32)
        wts = np.zeros((128, EL), np.float32)
        for le in range(EL):
            e = c * EL + le
            n = len(toks[e])
            if n:
                # [hk, 128, n] -> [128, hk, n]
                xg[:, le, :, :n] = xt[:, :, toks[e]].transpose(1, 0, 2)
                wts[:n, le] = wvals[e]
        # pre-tile weights to the SBUF layout (one 22.5KB contiguous line per
        # partition per chunk): w1t[le,hc,p,j*I+i] = w1[e, hc*1024+j*128+p, i]
        w1t = (w1[c * EL:(c + 1) * EL].astype(npdt)
               .reshape(EL, 2, 8, 128, I).transpose(0, 1, 3, 2, 4)
               .reshape(EL, 2, 128, 8 * I))
        # w2t[le,hh,p,ik*1024+u] = w2[e, ik*128+p, hh*1024+u]
        w2t = (w2[c * EL:(c + 1) * EL].astype(npdt)
               .reshape(EL, IT, 128, 2, 1024).transpose(0, 3, 2, 1, 4)
               .reshape(EL, 2, 128, IT * 1024))
        in_maps.append({
            "xgt": np.ascontiguousarray(
                xg.reshape(128, EL * HK * cap).astype(npdt)),
            "wts": np.ascontiguousarray(wts),
            "w1": np.ascontiguousarray(w1t),
            "w2": np.ascontiguousarray(w2t),
        })
    return in_maps


_NC_CACHE = {}


def _get_nc(mdt=MDT, cap=CAP):
    key = (mdt, cap)
    if key not in _NC_CACHE:
        _NC_CACHE[key] = build(mdt, cap)
    return _NC_CACHE[key]


def kernel(x, gate_w, w1, w2, topk=TOPK, **_):
    assert int(topk) == TOPK
    routing = route(x, gate_w)
    toks, _w = routing
    maxload = max(len(v) for v in toks)
    cap = CAP if maxload <= CAP else (maxload + 7) // 8 * 8
    nc = _get_nc(cap=cap)
    in_maps = make_in_maps(x, gate_w, w1, w2, cap=cap, routing=routing)
    res = run_bass_kernel_spmd(nc, in_maps, core_ids=list(range(NCORES)))
    out = np.zeros((T, H), np.float32)
    for c in range(NCORES):
        y = np.asarray(res.results[c]["y"], np.float32)   # [EL, cap, H]
        for le in range(EL):
            e = c * EL + le
            n = len(toks[e])
            if n:
                out[toks[e]] += y[le, :n]
    return out
